# revision 38
# baseline (speedup 1.0000x reference)
"""Causal self-attention (GQA + RoPE) on 8 Trainium2 NeuronCores.

Sharding: data-parallel over batch (2) x tensor-parallel over KV-head groups
(4).  Core c handles batch b=c//4 and KV group g=c%4 (Q heads 4g..4g+3).
Each core computes qkv^T in a transposed [d, t] layout, runs attention with
scores in S^T[k, q] orientation (no transposes needed), then computes a
PARTIAL output projection over its local 512 channels for all 2048 output
columns (contraction-sharded c_proj) and ReduceScatters the fp32 partials
per 512-token chunk straight into the output rows it owns.  This replaces
the old AllGather(y) + column-sharded c_proj: the collective output shrinks
4x ([128,2048] fp32 vs [2048,512] bf16 x4), cutting serialized collective
time from ~270us to ~165us and overlapping it with attention compute.
Matmuls run in fp32r for scores and bf16 elsewhere.
"""

import sys

if "/opt/trn_rl_repo" not in sys.path:
    sys.path.insert(0, "/opt/trn_rl_repo")

import numpy as np

# Problem constants (hardcoded per contract)
B, T, C = 2, 2048, 2048
H, KV, HD = 16, 4, 128
G = 4               # TP groups (KV heads) per batch
N_CORES = 8
N_HL = H // KV      # local Q heads per core = 4
ROPE_THETA = 10000.0
TC = 512            # free-dim chunk for moving operands
NT = T // TC        # 4 t-chunks
# ReduceScatter row-segments per chunk (last chunk ends with a small piece
# so the final serialized collective+DMA tail is short)
RS_SEGS = [[256, 256], [256, 256], [256, 256], [384, 128]]
NCT = C // 128      # 16 contraction tiles
NKT = T // 128      # 16 key tiles
SCALE = float(1.0 / np.sqrt(HD))

_COMPILED = {}


def _build(masked: bool):
    import concourse.bacc as bacc
    import concourse.tile as tile
    import concourse.mybir as mybir

    f32 = mybir.dt.float32

    nc = bacc.Bacc("TRN2", target_bir_lowering=False, debug=False,
                   num_devices=N_CORES, num_swdge_queues=4)

    bf16 = mybir.dt.bfloat16
    xT = nc.dram_tensor("xT", [C, T], bf16, kind="ExternalInput").ap()
    wqkvT = nc.dram_tensor("wqkvT", [C, (N_HL + 2) * HD], bf16,
                           kind="ExternalInput").ap()
    # local channel rows of w_proj^T: [512 ch, 2048 out]
    wpT = nc.dram_tensor("wpT", [TC, C], bf16, kind="ExternalInput").ap()
    cosT = nc.dram_tensor("cosT", [HD, T], f32, kind="ExternalInput").ap()
    sinT = nc.dram_tensor("sinT", [HD, T], f32, kind="ExternalInput").ap()
    binmask = nc.dram_tensor("binmask", [128, G * TC], bf16,
                             kind="ExternalInput").ap()
    kmask = None
    if masked:
        kmask = nc.dram_tensor("kmask", [128, NKT], f32,
                               kind="ExternalInput").ap()
    # 4 chunks x 128 token rows owned by this core, all 2048 columns
    out = nc.dram_tensor("out", [NT * 128, C], bf16, kind="ExternalOutput").ap()

    with tile.TileContext(nc, pool_alloc_mode="queue") as tc, \
         nc.allow_low_precision(reason="fp32r intermediates are intentional"):
        _build_body(nc, tc, mybir, f32,
                    xT, wqkvT, wpT, cosT, sinT, binmask, kmask, out)
    nc.compile()
    return nc


def _build_body(nc, tc, mybir, f32,
                xT, wqkvT, wpT, cosT, sinT, binmask, kmask, out):
    bf16 = mybir.dt.bfloat16
    f32r = mybir.dt.float32r
    from contextlib import ExitStack
    from concourse.masks import make_identity

    AF = mybir.ActivationFunctionType
    NR = N_HL + 2  # 6 row-tiles of qkv^T (4 q heads, k, v)
    NQ = TC // 128  # 4 128-subtiles per chunk

    with ExitStack() as ctx:
        # ---- pools (all share the kernel lifetime; queue allocator) ----
        const = ctx.enter_context(tc.tile_pool(name="const", bufs=1))
        rqkv = ctx.enter_context(tc.tile_pool(name="rqkv", bufs=1))
        dram = ctx.enter_context(tc.tile_pool(name="dram", bufs=1, space="DRAM"))
        raw_pool = ctx.enter_context(tc.tile_pool(name="raw_pool", bufs=3))
        rope_tmp = ctx.enter_context(tc.tile_pool(name="rope_tmp", bufs=3))
        pt_pool = ctx.enter_context(tc.tile_pool(name="pt_pool", bufs=6))
        norm_pool = ctx.enter_context(tc.tile_pool(name="norm_pool", bufs=3))
        yt_pool = ctx.enter_context(tc.tile_pool(name="yt_pool", bufs=2))
        o_sb_pool = ctx.enter_context(tc.tile_pool(name="o_sb", bufs=4))
        # softmax-sum accumulators (p tiles summed across kt on DVE/Pool)
        sacc_pool = ctx.enter_context(tc.tile_pool(name="sacc", bufs=4))
        # PSUM: 6 + 2 = 8 banks
        mm_psum = ctx.enter_context(
            tc.tile_pool(name="mm_psum", bufs=6, space="PSUM"))
        y_psum = ctx.enter_context(
            tc.tile_pool(name="y_psum", bufs=2, space="PSUM"))

        ident = const.tile([128, 128], f32)
        make_identity(nc, ident[:])
        ones_sq_f = const.tile([128, 128], f32)
        nc.vector.memset(ones_sq_f[:], 1.0)
        ones_sq = const.tile([128, 128], bf16)
        nc.vector.tensor_copy(ones_sq[:], ones_sq_f[:])
        # mask tiles allocated here; DMAs issued after the critical-path
        # loads (first needed at phase_b(0), ~100us in)
        mask_sb = const.tile([128, G * TC], bf16)
        kmask_sb = None
        if kmask is not None:
            kmask_sb = const.tile([128, NKT], f32)

        # resident activations (rotated q/k in [d, t] layout, v in [t, d])
        rq = [rqkv.tile([HD, T], f32r, tag=f"rq{h}", name=f"rq{h}")
              for h in range(N_HL)]
        rk = rqkv.tile([HD, T], f32r, tag="rk")
        v_sb = [rqkv.tile([128, HD], bf16, tag=f"v{i}", name=f"v{i}")
                for i in range(NKT)]

        # ReduceScatter staging, one pair per c_proj token slice.  Chunks
        # 0-2 use two halves [256, C]; the last chunk splits the second half
        # into two quarters [128, C] so the final (fully serialized) RS and
        # out-DMA are as small as possible.
        def rs_tiles(j):
            rows = RS_SEGS[j]
            ins, outs = [], []
            for hh, r in enumerate(rows):
                ins.append(dram.tile([r, C], bf16, tag=f"rsi{j}_{hh}",
                                     name=f"rsi{j}_{hh}"))
                outs.append(dram.tile([r // G, C], bf16, tag=f"rso{j}_{hh}",
                                      name=f"rso{j}_{hh}"))
            return ins, outs, rows
        _rs = [rs_tiles(j) for j in range(NT)]
        rs_in = [t[0] for t in _rs]
        rs_out = [t[1] for t in _rs]
        rs_rows = [t[2] for t in _rs]

        # ============ Phase A: qkv^T projection + RoPE + V transpose ============
        def phase_a(j, wq, xt_pool, cos_sb, sin_sb, xt_qs, pre=None):
            if pre is not None:
                xt_tiles = pre
            else:
                xt_tiles = []
                for ct in range(NCT):
                    xt_t = xt_pool.tile([128, TC], bf16, tag=f"xt{ct}",
                                        name=f"xt{ct}_{j}")
                    xt_qs[ct % len(xt_qs)].dma_start(
                        xt_t[:], xT[128 * ct:128 * (ct + 1),
                                    TC * j:TC * (j + 1)])
                    xt_tiles.append(xt_t)
            for r in range(NR):
                ps = mm_psum.tile([128, TC], f32, tag="mm", name=f"qkv{r}_{j}")
                for ct in range(NCT):
                    nc.tensor.matmul(ps[:],
                                     wq[ct][:, 128 * r:128 * (r + 1)],
                                     xt_tiles[ct][:],
                                     start=(ct == 0), stop=(ct == NCT - 1))
                if r < N_HL + 1:
                    # RoPE straight off PSUM:
                    #   dst = ps*cos + rot_half(ps)*sin_signed
                    dst = (rq[r] if r < N_HL else rk)[:, TC * j:TC * (j + 1)]
                    cs = cos_sb[:, TC * j:TC * (j + 1)]
                    sn = sin_sb[:, TC * j:TC * (j + 1)]
                    t1 = rope_tmp.tile([128, TC], f32, tag="t1",
                                       name=f"t1_{r}_{j}")
                    nc.vector.tensor_mul(t1[:], ps[:], cs[:])
                    t2 = rope_tmp.tile([128, TC], f32, tag="t2",
                                       name=f"t2_{r}_{j}")
                    nc.vector.tensor_mul(t2[0:64, :], ps[64:128, :], sn[0:64, :])
                    nc.vector.tensor_mul(t2[64:128, :], ps[0:64, :], sn[64:128, :])
                    nc.vector.tensor_add(dst, t1[:], t2[:])
                else:
                    # V: evict then transpose v^T [d, t] -> v [t, d]
                    rt = raw_pool.tile([128, TC], f32, tag="raw",
                                       name=f"vraw{j}")
                    nc.vector.tensor_copy(rt[:], ps[:])
                    for q in range(NQ):
                        pt = mm_psum.tile([128, 128], f32, tag="mm",
                                          name=f"vt{j}_{q}")
                        nc.tensor.transpose(
                            pt[:], rt[:, 128 * q:128 * (q + 1)], ident[:])
                        nc.vector.tensor_copy(v_sb[j * NQ + q][:], pt[:])

        # ============ Phase B: attention for q-chunk j ============
        def phase_b(j):
            nkt = (j + 1) * NQ  # causal limit in 128-k tiles
            ps_y = {}
            acc = {}
            pt = {}
            yts = {}

            def emit_scores(h, kt):
                # diagonal k-tile r>0: queries q < 128r in this chunk lie
                # strictly above the causal boundary -> p==0 there; skip
                # computing those columns entirely.
                r = kt - NQ * j
                c0 = 128 * r if r > 0 else 0
                ps_s = mm_psum.tile([128, TC], f32, tag="mm",
                                    name=f"s{h}_{j}_{kt}")
                nc.tensor.matmul(ps_s[:, c0:TC],
                                 rk[:, 128 * kt:128 * (kt + 1)],
                                 rq[h][:, TC * j + c0:TC * (j + 1)],
                                 start=True, stop=True)
                p = pt_pool.tile([128, TC], bf16, tag="pt",
                                 name=f"pt{h}_{j}_{kt}")
                nc.scalar.activation(p[:, c0:TC], ps_s[:, c0:TC],
                                     AF.Exp, scale=SCALE)
                if r >= 0:
                    nc.vector.tensor_mul(
                        p[:, c0:TC], p[:, c0:TC],
                        mask_sb[:, TC * r + c0:TC * (r + 1)])
                if kmask_sb is not None:
                    nc.vector.tensor_scalar_mul(
                        p[:, c0:TC], p[:, c0:TC], kmask_sb[:, kt:kt + 1])
                pt[(h, kt)] = (p, c0)

            for hp in range(N_HL // 2):
                pair = (2 * hp, 2 * hp + 1)
                units = [(h, kt) for kt in range(nkt) for h in pair]
                for k in range(min(3, len(units))):
                    emit_scores(*units[k])
                for idx, (h, kt) in enumerate(units):
                    if idx + 3 < len(units):
                        emit_scores(*units[idx + 3])
                    if kt == 0:
                        ps_y[h] = y_psum.tile([HD, TC], f32, tag="y",
                                              name=f"y{h}_{j}")
                        acc[h] = sacc_pool.tile([128, TC], bf16,
                                                tag=f"acc{h}",
                                                name=f"acc{h}_{j}")
                    p, c0 = pt.pop((h, kt))
                    # softmax denominator: accumulate p tiles across kt on
                    # DVE/Pool instead of one PE matmul per tile; a single
                    # ones-matmul per (h, chunk) reduces over k at the end
                    eng = nc.vector if h % 2 == 0 else nc.gpsimd
                    if kt == 0:
                        eng.tensor_copy(acc[h][:], p[:])
                    else:
                        eng.tensor_add(acc[h][:, c0:TC], acc[h][:, c0:TC],
                                       p[:, c0:TC])
                    nc.tensor.matmul(ps_y[h][:, c0:TC], v_sb[kt][:],
                                     p[:, c0:TC],
                                     start=(kt == 0), stop=(kt == nkt - 1),
                                     skip_group_check=True)
                    if kt != nkt - 1:
                        continue
                    ps_sum = mm_psum.tile([128, TC], f32, tag="mm",
                                          name=f"sum{h}_{j}")
                    nc.tensor.matmul(ps_sum[:], ones_sq[:], acc[h][:],
                                     start=True, stop=True)
                    # normalize straight off PSUM: fast approx reciprocal of
                    # the exp-sum, then scale the y accumulator while
                    # converting to bf16 (frees both banks immediately after)
                    rsum = norm_pool.tile([HD, TC], f32, tag="rsum",
                                          name=f"rs{h}_{j}")
                    nc.vector.reciprocal_approx_fast(rsum[:], ps_sum[:])
                    yt_t = yt_pool.tile([HD, TC], bf16, tag=f"yt{h}",
                                        name=f"yt{h}_{j}")
                    nc.vector.tensor_mul(yt_t[:], ps_y[h][:], rsum[:])
                    yts[h] = yt_t
            return yts

        # ====== Phase P: partial c_proj over local 512 channels (chunk j) ======
        def phase_p(j, yts, wp):
            # rs_in[j][t, o] = sum_h yt[h]^T @ wp[h]  (contraction over 512 ch)
            seg_starts = []
            s = 0
            for r in rs_rows[j]:
                seg_starts.append(s)
                s += r

            def seg_of(tt):
                for si in reversed(range(len(seg_starts))):
                    if 128 * tt >= seg_starts[si]:
                        return si
                return 0

            for tt in range(NQ):
                if tt > 0 and seg_of(tt) != seg_of(tt - 1):
                    phase_rs(j, seg_of(tt - 1))
                for ob in range(NQ):
                    # later groups borrow the idle attention accumulator
                    # banks so mm_psum drains before the next chunk's
                    # score matmuls need it
                    gi = tt * NQ + ob
                    if gi < 12:
                        ps = mm_psum.tile([128, TC], f32, tag="mm",
                                          name=f"pp{j}_{tt}_{ob}")
                    else:
                        ps = y_psum.tile([HD, TC], f32, tag="y",
                                         name=f"pp{j}_{tt}_{ob}")
                    for h in range(N_HL):
                        nc.tensor.matmul(
                            ps[:],
                            yts[h][:, 128 * tt:128 * (tt + 1)],
                            wp[h][:, TC * ob:TC * (ob + 1)],
                            start=(h == 0), stop=(h == N_HL - 1))
                    ev = o_sb_pool.tile([128, TC], bf16, tag="ev",
                                        name=f"ev{j}_{tt}_{ob}")
                    # mostly-DVE evictions: ACT eviction work sits ahead of
                    # the next chunk's softmax exps in the ACT queue, so
                    # keep its share small
                    if gi % 4 == 1:
                        nc.scalar.activation(ev[:], ps[:], AF.Copy)
                    else:
                        nc.vector.tensor_copy(ev[:], ps[:])
                    si = seg_of(tt)
                    rr = 128 * tt - seg_starts[si]
                    nc.sync.dma_start(
                        rs_in[j][si][rr:rr + 128, TC * ob:TC * (ob + 1)],
                        ev[:])

        def phase_rs(j, hh):
            # trigger-only: the out-DMA bounce happens in the epilogue so
            # the gpsimd queue never blocks a later collective trigger
            # behind a DMA that waits on an earlier collective
            nc.gpsimd.collective_compute(
                "ReduceScatter",
                mybir.AluOpType.add,
                replica_groups=[[0, 1, 2, 3], [4, 5, 6, 7]],
                ins=[rs_in[j][hh].opt()],
                outs=[rs_out[j][hh].opt()],
            )

        def epilogue():
            # collectives cannot write IO tensors; bounce DRAM->DRAM.
            # Ready pieces (chunks 0..NT-2) first, then the still-in-flight
            # chunk NT-1 pieces, spread over all three queues so each queue
            # ends on at most one waiting DMA.
            qs = [nc.gpsimd, nc.sync, nc.scalar]
            items = []
            for j in range(NT):
                for hh in range(len(rs_rows[j])):
                    seg0 = sum(rs_rows[j][:hh])
                    nrow = rs_rows[j][hh] // G
                    r0 = 128 * j + seg0 // G
                    items.append((j, hh, r0, nrow))
            qi = 0
            for j, hh, r0, nrow in items:
                last = (j == NT - 1) and (hh == len(rs_rows[j]) - 1)
                if not last:
                    qs[qi % 3].dma_start(out[r0:r0 + nrow, :],
                                         rs_out[j][hh][:])
                    qi += 1
                else:
                    nc.sync.dma_start(out[r0:r0 + nrow // 2, :],
                                      rs_out[j][hh][0:nrow // 2, :])
                    nc.scalar.dma_start(out[r0 + nrow // 2:r0 + nrow, :],
                                        rs_out[j][hh][nrow // 2:nrow, :])

        with tc.tile_pool(name="wq_pool", bufs=1) as wq_pool, \
             tc.tile_pool(name="xt_pool", bufs=2) as xt_pool, \
             tc.tile_pool(name="cs_pool", bufs=1) as cs_pool, \
             tc.tile_pool(name="wp_pool", bufs=1) as wp_pool:
            cos_sb = cs_pool.tile([HD, T], f32)
            sin_sb = cs_pool.tile([HD, T], f32)
            wq = [wq_pool.tile([128, NR * HD], bf16, tag=f"wq{ct}",
                               name=f"wq{ct}")
                  for ct in range(NCT)]
            # critical path: first matmuls need wq[ct] + chunk-0 xt[ct] in ct
            # order.  wq rides the two HWDGE queues; chunk-0 xt tiles ride
            # the 4-wide SWDGE path.
            for ct in range(NCT):
                (nc.scalar if ct % 2 == 0 else nc.sync).dma_start(
                    wq[ct][:], wqkvT[128 * ct:128 * (ct + 1), :])
            # RoPE tables: chunk-0 slice right behind wq (first read after
            # the r=0 qkv chain); remaining slices after phase_a(0)
            nc.scalar.dma_start(cos_sb[:, 0:TC], cosT[:, 0:TC])
            nc.sync.dma_start(sin_sb[:, 0:TC], sinT[:, 0:TC])
            wp = [wp_pool.tile([128, C], bf16, tag=f"wp{h}", name=f"wp{h}")
                  for h in range(N_HL)]
            phase_a(0, wq, xt_pool, cos_sb, sin_sb, [nc.gpsimd])
            for j in range(1, NT):
                nc.scalar.dma_start(cos_sb[:, TC * j:TC * (j + 1)],
                                    cosT[:, TC * j:TC * (j + 1)])
                nc.sync.dma_start(sin_sb[:, TC * j:TC * (j + 1)],
                                  sinT[:, TC * j:TC * (j + 1)])
            for h in range(N_HL):
                (nc.scalar if h % 2 == 0 else nc.sync).dma_start(
                    wp[h][:], wpT[128 * h:128 * (h + 1), :])
            nc.gpsimd.dma_start(mask_sb[:], binmask[:])
            if kmask is not None:
                nc.gpsimd.dma_start(kmask_sb[:], kmask[:])
            # Interleave remaining qkv chunks with attention: while B(j)'s
            # matmuls run, A(j+2)'s xt tiles stream in, so qkv is never
            # DMA-throttled (keeps the PE p-state at full clock).
            qs3 = [nc.scalar, nc.sync, nc.gpsimd]

            def do_a(j):
                phase_a(j, wq, xt_pool, cos_sb, sin_sb,
                        [qs3[jj % 3] for jj in range(j, j + 3)])

            def do_bp(j):
                yts = phase_b(j)
                phase_p(j, yts, wp)
                phase_rs(j, len(rs_rows[j]) - 1)

            do_a(1)
            do_bp(0)
            do_a(2)
            do_bp(1)
            do_a(3)
            do_bp(2)
            do_bp(3)
            epilogue()


def _rope_tables():
    inv_freq = 1.0 / (ROPE_THETA ** (np.arange(0, HD, 2, dtype=np.float32) / HD))
    pos = np.arange(T, dtype=np.float32)
    freqs = pos[:, None] * inv_freq[None, :]
    emb = np.concatenate([freqs, freqs], axis=-1)          # [T, HD]
    cos = np.ascontiguousarray(np.cos(emb).astype(np.float32).T)   # [HD, T]
    sin = np.ascontiguousarray(np.sin(emb).astype(np.float32).T)
    sin[:64, :] *= -1.0                                    # sign for rotate_half
    return cos, sin


def _binmask():
    kk = np.arange(128)[:, None]
    qq = np.arange(TC)[None, :]
    blocks = [(kk <= qq - 128 * r).astype(np.float32) for r in range(G)]
    return np.ascontiguousarray(np.concatenate(blocks, axis=1))  # [128, 4*512]


def kernel(x, attention_mask, w_qkv, w_proj):
    from concourse.bass_utils import run_bass_kernel_spmd

    x = np.asarray(x, dtype=np.float32)
    attention_mask = np.asarray(attention_mask, dtype=np.float32)
    w_qkv = np.asarray(w_qkv, dtype=np.float32)
    w_proj = np.asarray(w_proj, dtype=np.float32)

    masked = not bool((attention_mask == 1.0).all())
    if masked:
        attention_mask = (attention_mask != 0.0).astype(np.float32)

    if masked not in _COMPILED:
        _COMPILED[masked] = _build(masked)
    nc = _COMPILED[masked]

    import ml_dtypes
    bf = ml_dtypes.bfloat16
    cos, sin = _rope_tables()
    bm = _binmask().astype(bf)
    wpT_full = np.ascontiguousarray(w_proj.T)              # [c, o]

    in_maps = []
    for c in range(N_CORES):
        b, g = divmod(c, G)
        xT_b = np.ascontiguousarray(x[b].T).astype(bf)
        rows_q = w_qkv[512 * g:512 * (g + 1)]
        rows_k = w_qkv[H * HD + HD * g:H * HD + HD * (g + 1)]
        rows_v = w_qkv[(H + KV) * HD + HD * g:(H + KV) * HD + HD * (g + 1)]
        wqkvT_g = np.ascontiguousarray(
            np.concatenate([rows_q, rows_k, rows_v], axis=0).T).astype(bf)
        m = {"xT": xT_b,
             "wqkvT": wqkvT_g,
             "wpT": np.ascontiguousarray(
                 wpT_full[TC * g:TC * (g + 1), :]).astype(bf),
             "cosT": cos, "sinT": sin, "binmask": bm}
        if masked:
            m["kmask"] = np.ascontiguousarray(
                attention_mask[b].reshape(NKT, 128).T)
        in_maps.append(m)

    trace = bool(globals().get("_TRACE", False))
    res = run_bass_kernel_spmd(nc, in_maps, core_ids=list(range(N_CORES)),
                               trace=trace)
    globals()["_LAST_RESULT"] = res

    y = np.empty((B, T, C), dtype=np.float32)
    for c in range(N_CORES):
        b, g = divmod(c, G)
        o = np.asarray(res.results[c]["out"], dtype=np.float32)  # [512, 2048]
        for j in range(NT):
            segs = RS_SEGS[j]
            s0 = 0
            for seg in segs:
                nrow = seg // G
                r0 = TC * j + s0 + nrow * g
                k0 = 128 * j + s0 // G
                y[b, r0:r0 + nrow, :] = o[k0:k0 + nrow, :]
                s0 += seg
    return y



# revision 41
# speedup vs baseline: 1.1340x; 1.1340x over previous
"""Causal self-attention (GQA + RoPE) on 8 Trainium2 NeuronCores.

Sharding: data-parallel over batch (2) x tensor-parallel over KV-head groups
(4).  Core c handles batch b=c//4 and KV group g=c%4 (Q heads 4g..4g+3).
Each core computes qkv^T in a transposed [d, t] layout, runs attention with
scores in S^T[k, q] orientation (no transposes needed), then computes a
PARTIAL output projection over its local 512 channels for all 2048 output
columns (contraction-sharded c_proj) and ReduceScatters the fp32 partials
per 512-token chunk straight into the output rows it owns.  This replaces
the old AllGather(y) + column-sharded c_proj: the collective output shrinks
4x ([128,2048] fp32 vs [2048,512] bf16 x4), cutting serialized collective
time from ~270us to ~165us and overlapping it with attention compute.
Matmuls run in fp32r for scores and bf16 elsewhere.
"""

import sys

if "/opt/trn_rl_repo" not in sys.path:
    sys.path.insert(0, "/opt/trn_rl_repo")

import numpy as np

# Problem constants (hardcoded per contract)
B, T, C = 2, 2048, 2048
H, KV, HD = 16, 4, 128
G = 4               # TP groups (KV heads) per batch
N_CORES = 8
N_HL = H // KV      # local Q heads per core = 4
ROPE_THETA = 10000.0
TC = 512            # free-dim chunk for moving operands
NT = T // TC        # 4 t-chunks
# ReduceScatter row-segments per chunk (last chunk ends with a small piece
# so the final serialized collective+DMA tail is short)
RS_SEGS = [[256, 256], [256, 256], [256, 256], [384, 128]]
NCT = C // 128      # 16 contraction tiles
NKT = T // 128      # 16 key tiles
SCALE = float(1.0 / np.sqrt(HD))

_COMPILED = {}


def _build(masked: bool):
    import concourse.bacc as bacc
    import concourse.tile as tile
    import concourse.mybir as mybir

    f32 = mybir.dt.float32

    nc = bacc.Bacc("TRN2", target_bir_lowering=False, debug=False,
                   num_devices=N_CORES, num_swdge_queues=4)

    bf16 = mybir.dt.bfloat16
    xT = nc.dram_tensor("xT", [C, T], bf16, kind="ExternalInput").ap()
    wqkvT = nc.dram_tensor("wqkvT", [C, (N_HL + 2) * HD], bf16,
                           kind="ExternalInput").ap()
    # local channel rows of w_proj^T: [512 ch, 2048 out]
    wpT = nc.dram_tensor("wpT", [TC, C], bf16, kind="ExternalInput").ap()
    cosT = nc.dram_tensor("cosT", [HD, T], f32, kind="ExternalInput").ap()
    sinT = nc.dram_tensor("sinT", [HD, T], f32, kind="ExternalInput").ap()
    binmask = nc.dram_tensor("binmask", [128, G * TC], bf16,
                             kind="ExternalInput").ap()
    kmask = None
    if masked:
        kmask = nc.dram_tensor("kmask", [128, NKT], f32,
                               kind="ExternalInput").ap()
    # 4 chunks x 128 token rows owned by this core, all 2048 columns
    out = nc.dram_tensor("out", [NT * 128, C], bf16, kind="ExternalOutput").ap()

    with tile.TileContext(nc, pool_alloc_mode="queue") as tc, \
         nc.allow_low_precision(reason="fp32r intermediates are intentional"):
        _build_body(nc, tc, mybir, f32,
                    xT, wqkvT, wpT, cosT, sinT, binmask, kmask, out)
    nc.compile()
    return nc


def _build_body(nc, tc, mybir, f32,
                xT, wqkvT, wpT, cosT, sinT, binmask, kmask, out):
    bf16 = mybir.dt.bfloat16
    f32r = mybir.dt.float32r
    from contextlib import ExitStack
    from concourse.masks import make_identity

    AF = mybir.ActivationFunctionType
    NR = N_HL + 2  # 6 row-tiles of qkv^T (4 q heads, k, v)
    NQ = TC // 128  # 4 128-subtiles per chunk

    with ExitStack() as ctx:
        # ---- pools (all share the kernel lifetime; queue allocator) ----
        const = ctx.enter_context(tc.tile_pool(name="const", bufs=1))
        rqkv = ctx.enter_context(tc.tile_pool(name="rqkv", bufs=1))
        dram = ctx.enter_context(tc.tile_pool(name="dram", bufs=1, space="DRAM"))
        raw_pool = ctx.enter_context(tc.tile_pool(name="raw_pool", bufs=3))
        rope_tmp = ctx.enter_context(tc.tile_pool(name="rope_tmp", bufs=3))
        pt_pool = ctx.enter_context(tc.tile_pool(name="pt_pool", bufs=6))
        norm_pool = ctx.enter_context(tc.tile_pool(name="norm_pool", bufs=3))
        yt_pool = ctx.enter_context(tc.tile_pool(name="yt_pool", bufs=2))
        o_sb_pool = ctx.enter_context(tc.tile_pool(name="o_sb", bufs=4))
        # PSUM: 4 + 2 + 2 = 8 banks
        mm_psum = ctx.enter_context(
            tc.tile_pool(name="mm_psum", bufs=4, space="PSUM"))
        y_psum = ctx.enter_context(
            tc.tile_pool(name="y_psum", bufs=2, space="PSUM"))
        sum_psum = ctx.enter_context(
            tc.tile_pool(name="sum_psum", bufs=2, space="PSUM"))

        ident = const.tile([128, 128], f32)
        make_identity(nc, ident[:])
        ones_sq_f = const.tile([128, 128], f32)
        nc.vector.memset(ones_sq_f[:], 1.0)
        ones_sq = const.tile([128, 128], bf16)
        nc.vector.tensor_copy(ones_sq[:], ones_sq_f[:])
        # mask tiles allocated here; DMAs issued after the critical-path
        # loads (first needed at phase_b(0), ~100us in)
        mask_sb = const.tile([128, G * TC], bf16)
        kmask_sb = None
        if kmask is not None:
            kmask_sb = const.tile([128, NKT], f32)

        # resident activations (rotated q/k in [d, t] layout, v in [t, d])
        rq = [rqkv.tile([HD, T], f32r, tag=f"rq{h}", name=f"rq{h}")
              for h in range(N_HL)]
        rk = rqkv.tile([HD, T], f32r, tag="rk")
        v_sb = [rqkv.tile([128, HD], bf16, tag=f"v{i}", name=f"v{i}")
                for i in range(NKT)]

        # ReduceScatter staging, one pair per c_proj token slice.  Chunks
        # 0-2 use two halves [256, C]; the last chunk splits the second half
        # into two quarters [128, C] so the final (fully serialized) RS and
        # out-DMA are as small as possible.
        def rs_tiles(j):
            rows = RS_SEGS[j]
            ins, outs = [], []
            for hh, r in enumerate(rows):
                ins.append(dram.tile([r, C], bf16, tag=f"rsi{j}_{hh}",
                                     name=f"rsi{j}_{hh}"))
                outs.append(dram.tile([r // G, C], bf16, tag=f"rso{j}_{hh}",
                                      name=f"rso{j}_{hh}"))
            return ins, outs, rows
        _rs = [rs_tiles(j) for j in range(NT)]
        rs_in = [t[0] for t in _rs]
        rs_out = [t[1] for t in _rs]
        rs_rows = [t[2] for t in _rs]

        # ============ Phase A: qkv^T projection + RoPE + V transpose ============
        def phase_a(j, wq, xt_pool, cos_sb, sin_sb, xt_qs, pre=None):
            if pre is not None:
                xt_tiles = pre
            else:
                xt_tiles = []
                for ct in range(NCT):
                    xt_t = xt_pool.tile([128, TC], bf16, tag=f"xt{ct}",
                                        name=f"xt{ct}_{j}")
                    xt_qs[ct % len(xt_qs)].dma_start(
                        xt_t[:], xT[128 * ct:128 * (ct + 1),
                                    TC * j:TC * (j + 1)])
                    xt_tiles.append(xt_t)
            for r in range(NR):
                ps = mm_psum.tile([128, TC], f32, tag="mm", name=f"qkv{r}_{j}")
                for ct in range(NCT):
                    nc.tensor.matmul(ps[:],
                                     wq[ct][:, 128 * r:128 * (r + 1)],
                                     xt_tiles[ct][:],
                                     start=(ct == 0), stop=(ct == NCT - 1))
                if r < N_HL + 1:
                    # RoPE straight off PSUM:
                    #   dst = ps*cos + rot_half(ps)*sin_signed
                    dst = (rq[r] if r < N_HL else rk)[:, TC * j:TC * (j + 1)]
                    cs = cos_sb[:, TC * j:TC * (j + 1)]
                    sn = sin_sb[:, TC * j:TC * (j + 1)]
                    t1 = rope_tmp.tile([128, TC], f32, tag="t1",
                                       name=f"t1_{r}_{j}")
                    nc.vector.tensor_mul(t1[:], ps[:], cs[:])
                    t2 = rope_tmp.tile([128, TC], f32, tag="t2",
                                       name=f"t2_{r}_{j}")
                    nc.vector.tensor_mul(t2[0:64, :], ps[64:128, :], sn[0:64, :])
                    nc.vector.tensor_mul(t2[64:128, :], ps[0:64, :], sn[64:128, :])
                    nc.vector.tensor_add(dst, t1[:], t2[:])
                else:
                    # V: evict then transpose v^T [d, t] -> v [t, d]
                    rt = raw_pool.tile([128, TC], f32, tag="raw",
                                       name=f"vraw{j}")
                    nc.vector.tensor_copy(rt[:], ps[:])
                    for q in range(NQ):
                        pt = mm_psum.tile([128, 128], f32, tag="mm",
                                          name=f"vt{j}_{q}")
                        nc.tensor.transpose(
                            pt[:], rt[:, 128 * q:128 * (q + 1)], ident[:])
                        nc.vector.tensor_copy(v_sb[j * NQ + q][:], pt[:])

        # ============ Phase B: attention for q-chunk j ============
        def phase_b(j):
            nkt = (j + 1) * NQ  # causal limit in 128-k tiles
            ps_y = {}
            acc = {}
            pt = {}
            yts = {}

            def emit_scores(h, kt):
                # diagonal k-tile r>0: queries q < 128r in this chunk lie
                # strictly above the causal boundary -> p==0 there; skip
                # computing those columns entirely.
                r = kt - NQ * j
                c0 = 128 * r if r > 0 else 0
                ps_s = mm_psum.tile([128, TC], f32, tag="mm",
                                    name=f"s{h}_{j}_{kt}")
                nc.tensor.matmul(ps_s[:, c0:TC],
                                 rk[:, 128 * kt:128 * (kt + 1)],
                                 rq[h][:, TC * j + c0:TC * (j + 1)],
                                 start=True, stop=True)
                p = pt_pool.tile([128, TC], bf16, tag="pt",
                                 name=f"pt{h}_{j}_{kt}")
                nc.scalar.activation(p[:, c0:TC], ps_s[:, c0:TC],
                                     AF.Exp, scale=SCALE)
                if r >= 0:
                    nc.vector.tensor_mul(
                        p[:, c0:TC], p[:, c0:TC],
                        mask_sb[:, TC * r + c0:TC * (r + 1)])
                if kmask_sb is not None:
                    nc.vector.tensor_scalar_mul(
                        p[:, c0:TC], p[:, c0:TC], kmask_sb[:, kt:kt + 1])
                pt[(h, kt)] = (p, c0)

            for hp in range(N_HL // 2):
                pair = (2 * hp, 2 * hp + 1)
                units = [(h, kt) for kt in range(nkt) for h in pair]
                emit_scores(*units[0])
                if len(units) > 1:
                    emit_scores(*units[1])
                for idx, (h, kt) in enumerate(units):
                    if idx + 2 < len(units):
                        emit_scores(*units[idx + 2])
                    if kt == 0:
                        ps_y[h] = y_psum.tile([HD, TC], f32, tag="y",
                                              name=f"y{h}_{j}")
                        acc[h] = sum_psum.tile([128, TC], f32, tag="sum",
                                               name=f"sum{h}_{j}")
                    p, c0 = pt.pop((h, kt))
                    nc.tensor.matmul(acc[h][:, c0:TC], ones_sq[:],
                                     p[:, c0:TC],
                                     start=(kt == 0), stop=(kt == nkt - 1),
                                     skip_group_check=True)
                    nc.tensor.matmul(ps_y[h][:, c0:TC], v_sb[kt][:],
                                     p[:, c0:TC],
                                     start=(kt == 0), stop=(kt == nkt - 1),
                                     skip_group_check=True)
                    if kt != nkt - 1:
                        continue
                    # normalize straight off PSUM: fast approx reciprocal of
                    # the exp-sum, then scale the y accumulator while
                    # converting to bf16 (frees both banks immediately after)
                    rsum = norm_pool.tile([HD, TC], f32, tag="rsum",
                                          name=f"rs{h}_{j}")
                    nc.vector.reciprocal_approx_fast(rsum[:], acc[h][:])
                    yt_t = yt_pool.tile([HD, TC], bf16, tag=f"yt{h}",
                                        name=f"yt{h}_{j}")
                    nc.vector.tensor_mul(yt_t[:], ps_y[h][:], rsum[:])
                    yts[h] = yt_t
            return yts

        # ====== Phase P: partial c_proj over local 512 channels (chunk j) ======
        def phase_p(j, yts, wp):
            # rs_in[j][t, o] = sum_h yt[h]^T @ wp[h]  (contraction over 512 ch)
            seg_starts = []
            s = 0
            for r in rs_rows[j]:
                seg_starts.append(s)
                s += r

            def seg_of(tt):
                for si in reversed(range(len(seg_starts))):
                    if 128 * tt >= seg_starts[si]:
                        return si
                return 0

            for tt in range(NQ):
                if tt > 0 and seg_of(tt) != seg_of(tt - 1):
                    phase_rs(j, seg_of(tt - 1))
                for ob in range(NQ):
                    # later groups borrow the idle attention accumulator
                    # banks so mm_psum drains before the next chunk's
                    # score matmuls need it
                    gi = tt * NQ + ob
                    if gi < 8:
                        ps = mm_psum.tile([128, TC], f32, tag="mm",
                                          name=f"pp{j}_{tt}_{ob}")
                    elif gi < 12:
                        ps = y_psum.tile([HD, TC], f32, tag="y",
                                         name=f"pp{j}_{tt}_{ob}")
                    else:
                        ps = sum_psum.tile([128, TC], f32, tag="sum",
                                           name=f"pp{j}_{tt}_{ob}")
                    for h in range(N_HL):
                        nc.tensor.matmul(
                            ps[:],
                            yts[h][:, 128 * tt:128 * (tt + 1)],
                            wp[h][:, TC * ob:TC * (ob + 1)],
                            start=(h == 0), stop=(h == N_HL - 1))
                    ev = o_sb_pool.tile([128, TC], bf16, tag="ev",
                                        name=f"ev{j}_{tt}_{ob}")
                    # mostly-DVE evictions: ACT eviction work sits ahead of
                    # the next chunk's softmax exps in the ACT queue, so
                    # keep its share small
                    if gi % 4 == 1:
                        nc.scalar.activation(ev[:], ps[:], AF.Copy)
                    else:
                        nc.vector.tensor_copy(ev[:], ps[:])
                    si = seg_of(tt)
                    rr = 128 * tt - seg_starts[si]
                    nc.sync.dma_start(
                        rs_in[j][si][rr:rr + 128, TC * ob:TC * (ob + 1)],
                        ev[:])

        def phase_rs(j, hh):
            # trigger-only: the out-DMA bounce happens in the epilogue so
            # the gpsimd queue never blocks a later collective trigger
            # behind a DMA that waits on an earlier collective
            nc.gpsimd.collective_compute(
                "ReduceScatter",
                mybir.AluOpType.add,
                replica_groups=[[0, 1, 2, 3], [4, 5, 6, 7]],
                ins=[rs_in[j][hh].opt()],
                outs=[rs_out[j][hh].opt()],
            )

        def epilogue():
            # collectives cannot write IO tensors; bounce DRAM->DRAM.
            # Ready pieces (chunks 0..NT-2) first, then the still-in-flight
            # chunk NT-1 pieces, spread over all three queues so each queue
            # ends on at most one waiting DMA.
            qs = [nc.gpsimd, nc.sync, nc.scalar]
            items = []
            for j in range(NT):
                for hh in range(len(rs_rows[j])):
                    seg0 = sum(rs_rows[j][:hh])
                    nrow = rs_rows[j][hh] // G
                    r0 = 128 * j + seg0 // G
                    items.append((j, hh, r0, nrow))
            qi = 0
            for j, hh, r0, nrow in items:
                last = (j == NT - 1) and (hh == len(rs_rows[j]) - 1)
                if not last:
                    qs[qi % 3].dma_start(out[r0:r0 + nrow, :],
                                         rs_out[j][hh][:])
                    qi += 1
                else:
                    nc.sync.dma_start(out[r0:r0 + nrow // 2, :],
                                      rs_out[j][hh][0:nrow // 2, :])
                    nc.scalar.dma_start(out[r0 + nrow // 2:r0 + nrow, :],
                                        rs_out[j][hh][nrow // 2:nrow, :])

        with tc.tile_pool(name="wq_pool", bufs=1) as wq_pool, \
             tc.tile_pool(name="xt_pool", bufs=2) as xt_pool, \
             tc.tile_pool(name="cs_pool", bufs=1) as cs_pool, \
             tc.tile_pool(name="wp_pool", bufs=1) as wp_pool:
            cos_sb = cs_pool.tile([HD, T], f32)
            sin_sb = cs_pool.tile([HD, T], f32)
            wq = [wq_pool.tile([128, NR * HD], bf16, tag=f"wq{ct}",
                               name=f"wq{ct}")
                  for ct in range(NCT)]
            # critical path: first matmuls need wq[ct] + chunk-0 xt[ct] in ct
            # order.  wq rides the two HWDGE queues; chunk-0 xt tiles ride
            # the 4-wide SWDGE path.
            for ct in range(NCT):
                (nc.scalar if ct % 2 == 0 else nc.sync).dma_start(
                    wq[ct][:], wqkvT[128 * ct:128 * (ct + 1), :])
            # RoPE tables: chunk-0 slice right behind wq (first read after
            # the r=0 qkv chain); remaining slices after phase_a(0)
            nc.scalar.dma_start(cos_sb[:, 0:TC], cosT[:, 0:TC])
            nc.sync.dma_start(sin_sb[:, 0:TC], sinT[:, 0:TC])
            wp = [wp_pool.tile([128, C], bf16, tag=f"wp{h}", name=f"wp{h}")
                  for h in range(N_HL)]
            phase_a(0, wq, xt_pool, cos_sb, sin_sb, [nc.gpsimd])
            for j in range(1, NT):
                nc.scalar.dma_start(cos_sb[:, TC * j:TC * (j + 1)],
                                    cosT[:, TC * j:TC * (j + 1)])
                nc.sync.dma_start(sin_sb[:, TC * j:TC * (j + 1)],
                                  sinT[:, TC * j:TC * (j + 1)])
            for h in range(N_HL):
                (nc.scalar if h % 2 == 0 else nc.sync).dma_start(
                    wp[h][:], wpT[128 * h:128 * (h + 1), :])
            nc.gpsimd.dma_start(mask_sb[:], binmask[:])
            if kmask is not None:
                nc.gpsimd.dma_start(kmask_sb[:], kmask[:])
            # Interleave remaining qkv chunks with attention: while B(j)'s
            # matmuls run, A(j+2)'s xt tiles stream in, so qkv is never
            # DMA-throttled (keeps the PE p-state at full clock).
            qs3 = [nc.scalar, nc.sync, nc.gpsimd]

            def do_a(j):
                phase_a(j, wq, xt_pool, cos_sb, sin_sb,
                        [qs3[jj % 3] for jj in range(j, j + 3)])

            def do_bp(j):
                yts = phase_b(j)
                phase_p(j, yts, wp)
                phase_rs(j, len(rs_rows[j]) - 1)

            do_a(1)
            do_bp(0)
            do_a(2)
            do_bp(1)
            do_a(3)
            do_bp(2)
            do_bp(3)
            epilogue()


def _rope_tables():
    inv_freq = 1.0 / (ROPE_THETA ** (np.arange(0, HD, 2, dtype=np.float32) / HD))
    pos = np.arange(T, dtype=np.float32)
    freqs = pos[:, None] * inv_freq[None, :]
    emb = np.concatenate([freqs, freqs], axis=-1)          # [T, HD]
    cos = np.ascontiguousarray(np.cos(emb).astype(np.float32).T)   # [HD, T]
    sin = np.ascontiguousarray(np.sin(emb).astype(np.float32).T)
    sin[:64, :] *= -1.0                                    # sign for rotate_half
    return cos, sin


def _binmask():
    kk = np.arange(128)[:, None]
    qq = np.arange(TC)[None, :]
    blocks = [(kk <= qq - 128 * r).astype(np.float32) for r in range(G)]
    return np.ascontiguousarray(np.concatenate(blocks, axis=1))  # [128, 4*512]


def kernel(x, attention_mask, w_qkv, w_proj):
    from concourse.bass_utils import run_bass_kernel_spmd

    x = np.asarray(x, dtype=np.float32)
    attention_mask = np.asarray(attention_mask, dtype=np.float32)
    w_qkv = np.asarray(w_qkv, dtype=np.float32)
    w_proj = np.asarray(w_proj, dtype=np.float32)

    masked = not bool((attention_mask == 1.0).all())
    if masked:
        attention_mask = (attention_mask != 0.0).astype(np.float32)

    if masked not in _COMPILED:
        _COMPILED[masked] = _build(masked)
    nc = _COMPILED[masked]

    import ml_dtypes
    bf = ml_dtypes.bfloat16
    cos, sin = _rope_tables()
    bm = _binmask().astype(bf)
    wpT_full = np.ascontiguousarray(w_proj.T)              # [c, o]

    in_maps = []
    for c in range(N_CORES):
        b, g = divmod(c, G)
        xT_b = np.ascontiguousarray(x[b].T).astype(bf)
        rows_q = w_qkv[512 * g:512 * (g + 1)]
        rows_k = w_qkv[H * HD + HD * g:H * HD + HD * (g + 1)]
        rows_v = w_qkv[(H + KV) * HD + HD * g:(H + KV) * HD + HD * (g + 1)]
        wqkvT_g = np.ascontiguousarray(
            np.concatenate([rows_q, rows_k, rows_v], axis=0).T).astype(bf)
        m = {"xT": xT_b,
             "wqkvT": wqkvT_g,
             "wpT": np.ascontiguousarray(
                 wpT_full[TC * g:TC * (g + 1), :]).astype(bf),
             "cosT": cos, "sinT": sin, "binmask": bm}
        if masked:
            m["kmask"] = np.ascontiguousarray(
                attention_mask[b].reshape(NKT, 128).T)
        in_maps.append(m)

    trace = bool(globals().get("_TRACE", False))
    res = run_bass_kernel_spmd(nc, in_maps, core_ids=list(range(N_CORES)),
                               trace=trace)
    globals()["_LAST_RESULT"] = res

    y = np.empty((B, T, C), dtype=np.float32)
    for c in range(N_CORES):
        b, g = divmod(c, G)
        o = np.asarray(res.results[c]["out"], dtype=np.float32)  # [512, 2048]
        for j in range(NT):
            segs = RS_SEGS[j]
            s0 = 0
            for seg in segs:
                nrow = seg // G
                r0 = TC * j + s0 + nrow * g
                k0 = 128 * j + s0 // G
                y[b, r0:r0 + nrow, :] = o[k0:k0 + nrow, :]
                s0 += seg
    return y



# revision 42
# speedup vs baseline: 1.1363x; 1.0020x over previous
"""Causal self-attention (GQA + RoPE) on 8 Trainium2 NeuronCores.

Sharding: data-parallel over batch (2) x tensor-parallel over KV-head groups
(4).  Core c handles batch b=c//4 and KV group g=c%4 (Q heads 4g..4g+3).
Each core computes qkv^T in a transposed [d, t] layout, runs attention with
scores in S^T[k, q] orientation (no transposes needed), then computes a
PARTIAL output projection over its local 512 channels for all 2048 output
columns (contraction-sharded c_proj) and ReduceScatters the bf16 partials
per token-segment straight into the output rows it owns.

Schedule/engine notes (each validated against perfetto traces):
- qkv chunks interleave with attention chunks (A0 A1 B0 A2 B1 A3 B2 B3) so
  qkv is never DMA-throttled and the PE p-state stays at full clock.
- softmax normalization runs straight off PSUM: reciprocal_approx_fast on
  the exp-sum accumulator + one tensor_mul, no intermediate copies.
- ReduceScatter is segmented ([256,256] per chunk, [384,128] for the last
  chunk) and triggered as soon as its token rows are evicted; all
  DRAM->DRAM out-bounces are deferred to an epilogue so the gpsimd queue
  never blocks a later collective trigger behind a DMA waiting on an
  earlier collective.
- c_proj PSUM groups borrow the idle attention accumulator banks, and
  evictions go 3:1 DVE:ACT (ACT eviction work would sit ahead of the next
  chunk's softmax exps in the ACT queue).
Matmuls run in fp32r for scores and bf16 elsewhere.
"""

import sys

if "/opt/trn_rl_repo" not in sys.path:
    sys.path.insert(0, "/opt/trn_rl_repo")

import numpy as np

# Problem constants (hardcoded per contract)
B, T, C = 2, 2048, 2048
H, KV, HD = 16, 4, 128
G = 4               # TP groups (KV heads) per batch
N_CORES = 8
N_HL = H // KV      # local Q heads per core = 4
ROPE_THETA = 10000.0
TC = 512            # free-dim chunk for moving operands
NT = T // TC        # 4 t-chunks
# ReduceScatter row-segments per chunk (last chunk ends with a small piece
# so the final serialized collective+DMA tail is short)
RS_SEGS = [[256, 256], [256, 256], [256, 256], [384, 128]]
NCT = C // 128      # 16 contraction tiles
NKT = T // 128      # 16 key tiles
SCALE = float(1.0 / np.sqrt(HD))

_COMPILED = {}


def _build(masked: bool):
    import concourse.bacc as bacc
    import concourse.tile as tile
    import concourse.mybir as mybir

    f32 = mybir.dt.float32

    nc = bacc.Bacc("TRN2", target_bir_lowering=False, debug=False,
                   num_devices=N_CORES, num_swdge_queues=4)

    bf16 = mybir.dt.bfloat16
    xT = nc.dram_tensor("xT", [C, T], bf16, kind="ExternalInput").ap()
    wqkvT = nc.dram_tensor("wqkvT", [C, (N_HL + 2) * HD], bf16,
                           kind="ExternalInput").ap()
    # local channel rows of w_proj^T: [512 ch, 2048 out]
    wpT = nc.dram_tensor("wpT", [TC, C], bf16, kind="ExternalInput").ap()
    cosT = nc.dram_tensor("cosT", [HD, T], f32, kind="ExternalInput").ap()
    sinT = nc.dram_tensor("sinT", [HD, T], f32, kind="ExternalInput").ap()
    binmask = nc.dram_tensor("binmask", [128, G * TC], bf16,
                             kind="ExternalInput").ap()
    kmask = None
    if masked:
        kmask = nc.dram_tensor("kmask", [128, NKT], f32,
                               kind="ExternalInput").ap()
    # 4 chunks x 128 token rows owned by this core, all 2048 columns
    out = nc.dram_tensor("out", [NT * 128, C], bf16, kind="ExternalOutput").ap()

    with tile.TileContext(nc, pool_alloc_mode="queue") as tc, \
         nc.allow_low_precision(reason="fp32r intermediates are intentional"):
        _build_body(nc, tc, mybir, f32,
                    xT, wqkvT, wpT, cosT, sinT, binmask, kmask, out)
    nc.compile()
    return nc


def _build_body(nc, tc, mybir, f32,
                xT, wqkvT, wpT, cosT, sinT, binmask, kmask, out):
    bf16 = mybir.dt.bfloat16
    f32r = mybir.dt.float32r
    from contextlib import ExitStack
    from concourse.masks import make_identity

    AF = mybir.ActivationFunctionType
    NR = N_HL + 2  # 6 row-tiles of qkv^T (4 q heads, k, v)
    NQ = TC // 128  # 4 128-subtiles per chunk

    with ExitStack() as ctx:
        # ---- pools (all share the kernel lifetime; queue allocator) ----
        const = ctx.enter_context(tc.tile_pool(name="const", bufs=1))
        rqkv = ctx.enter_context(tc.tile_pool(name="rqkv", bufs=1))
        dram = ctx.enter_context(tc.tile_pool(name="dram", bufs=1, space="DRAM"))
        raw_pool = ctx.enter_context(tc.tile_pool(name="raw_pool", bufs=3))
        rope_tmp = ctx.enter_context(tc.tile_pool(name="rope_tmp", bufs=3))
        pt_pool = ctx.enter_context(tc.tile_pool(name="pt_pool", bufs=6))
        norm_pool = ctx.enter_context(tc.tile_pool(name="norm_pool", bufs=3))
        yt_pool = ctx.enter_context(tc.tile_pool(name="yt_pool", bufs=2))
        o_sb_pool = ctx.enter_context(tc.tile_pool(name="o_sb", bufs=4))
        # PSUM: 4 + 2 + 2 = 8 banks
        mm_psum = ctx.enter_context(
            tc.tile_pool(name="mm_psum", bufs=4, space="PSUM"))
        y_psum = ctx.enter_context(
            tc.tile_pool(name="y_psum", bufs=2, space="PSUM"))
        sum_psum = ctx.enter_context(
            tc.tile_pool(name="sum_psum", bufs=2, space="PSUM"))

        ident = const.tile([128, 128], f32)
        make_identity(nc, ident[:])
        ones_sq_f = const.tile([128, 128], f32)
        nc.vector.memset(ones_sq_f[:], 1.0)
        ones_sq = const.tile([128, 128], bf16)
        nc.vector.tensor_copy(ones_sq[:], ones_sq_f[:])
        # mask tiles allocated here; DMAs issued after the critical-path
        # loads (first needed at phase_b(0), ~100us in)
        mask_sb = const.tile([128, G * TC], bf16)
        kmask_sb = None
        if kmask is not None:
            kmask_sb = const.tile([128, NKT], f32)

        # resident activations (rotated q/k in [d, t] layout, v in [t, d])
        rq = [rqkv.tile([HD, T], f32r, tag=f"rq{h}", name=f"rq{h}")
              for h in range(N_HL)]
        rk = rqkv.tile([HD, T], f32r, tag="rk")
        v_sb = [rqkv.tile([128, HD], bf16, tag=f"v{i}", name=f"v{i}")
                for i in range(NKT)]

        # ReduceScatter staging, one pair per c_proj token slice.  Chunks
        # 0-2 use two halves [256, C]; the last chunk splits the second half
        # into two quarters [128, C] so the final (fully serialized) RS and
        # out-DMA are as small as possible.
        def rs_tiles(j):
            rows = RS_SEGS[j]
            ins, outs = [], []
            for hh, r in enumerate(rows):
                ins.append(dram.tile([r, C], bf16, tag=f"rsi{j}_{hh}",
                                     name=f"rsi{j}_{hh}"))
                outs.append(dram.tile([r // G, C], bf16, tag=f"rso{j}_{hh}",
                                      name=f"rso{j}_{hh}"))
            return ins, outs, rows
        _rs = [rs_tiles(j) for j in range(NT)]
        rs_in = [t[0] for t in _rs]
        rs_out = [t[1] for t in _rs]
        rs_rows = [t[2] for t in _rs]

        # ============ Phase A: qkv^T projection + RoPE + V transpose ============
        def phase_a(j, wq, xt_pool, cos_sb, sin_sb, xt_qs, pre=None):
            if pre is not None:
                xt_tiles = pre
            else:
                xt_tiles = []
                for ct in range(NCT):
                    xt_t = xt_pool.tile([128, TC], bf16, tag=f"xt{ct}",
                                        name=f"xt{ct}_{j}")
                    xt_qs[ct % len(xt_qs)].dma_start(
                        xt_t[:], xT[128 * ct:128 * (ct + 1),
                                    TC * j:TC * (j + 1)])
                    xt_tiles.append(xt_t)
            for r in range(NR):
                ps = mm_psum.tile([128, TC], f32, tag="mm", name=f"qkv{r}_{j}")
                for ct in range(NCT):
                    nc.tensor.matmul(ps[:],
                                     wq[ct][:, 128 * r:128 * (r + 1)],
                                     xt_tiles[ct][:],
                                     start=(ct == 0), stop=(ct == NCT - 1))
                if r < N_HL + 1:
                    # RoPE straight off PSUM:
                    #   dst = ps*cos + rot_half(ps)*sin_signed
                    dst = (rq[r] if r < N_HL else rk)[:, TC * j:TC * (j + 1)]
                    cs = cos_sb[:, TC * j:TC * (j + 1)]
                    sn = sin_sb[:, TC * j:TC * (j + 1)]
                    t1 = rope_tmp.tile([128, TC], f32, tag="t1",
                                       name=f"t1_{r}_{j}")
                    nc.vector.tensor_mul(t1[:], ps[:], cs[:])
                    t2 = rope_tmp.tile([128, TC], f32, tag="t2",
                                       name=f"t2_{r}_{j}")
                    nc.vector.tensor_mul(t2[0:64, :], ps[64:128, :], sn[0:64, :])
                    nc.vector.tensor_mul(t2[64:128, :], ps[0:64, :], sn[64:128, :])
                    nc.vector.tensor_add(dst, t1[:], t2[:])
                else:
                    # V: evict then transpose v^T [d, t] -> v [t, d]
                    rt = raw_pool.tile([128, TC], f32, tag="raw",
                                       name=f"vraw{j}")
                    nc.vector.tensor_copy(rt[:], ps[:])
                    for q in range(NQ):
                        pt = mm_psum.tile([128, 128], f32, tag="mm",
                                          name=f"vt{j}_{q}")
                        nc.tensor.transpose(
                            pt[:], rt[:, 128 * q:128 * (q + 1)], ident[:])
                        nc.vector.tensor_copy(v_sb[j * NQ + q][:], pt[:])

        # ============ Phase B: attention for q-chunk j ============
        def phase_b(j):
            nkt = (j + 1) * NQ  # causal limit in 128-k tiles
            ps_y = {}
            acc = {}
            pt = {}
            yts = {}

            def emit_scores(h, kt):
                # diagonal k-tile r>0: queries q < 128r in this chunk lie
                # strictly above the causal boundary -> p==0 there; skip
                # computing those columns entirely.
                r = kt - NQ * j
                c0 = 128 * r if r > 0 else 0
                ps_s = mm_psum.tile([128, TC], f32, tag="mm",
                                    name=f"s{h}_{j}_{kt}")
                nc.tensor.matmul(ps_s[:, c0:TC],
                                 rk[:, 128 * kt:128 * (kt + 1)],
                                 rq[h][:, TC * j + c0:TC * (j + 1)],
                                 start=True, stop=True)
                p = pt_pool.tile([128, TC], bf16, tag="pt",
                                 name=f"pt{h}_{j}_{kt}")
                nc.scalar.activation(p[:, c0:TC], ps_s[:, c0:TC],
                                     AF.Exp, scale=SCALE)
                if r >= 0:
                    nc.vector.tensor_mul(
                        p[:, c0:TC], p[:, c0:TC],
                        mask_sb[:, TC * r + c0:TC * (r + 1)])
                if kmask_sb is not None:
                    nc.vector.tensor_scalar_mul(
                        p[:, c0:TC], p[:, c0:TC], kmask_sb[:, kt:kt + 1])
                pt[(h, kt)] = (p, c0)

            for hp in range(N_HL // 2):
                pair = (2 * hp, 2 * hp + 1)
                units = [(h, kt) for kt in range(nkt) for h in pair]
                emit_scores(*units[0])
                if len(units) > 1:
                    emit_scores(*units[1])
                for idx, (h, kt) in enumerate(units):
                    if idx + 2 < len(units):
                        emit_scores(*units[idx + 2])
                    if kt == 0:
                        ps_y[h] = y_psum.tile([HD, TC], f32, tag="y",
                                              name=f"y{h}_{j}")
                        acc[h] = sum_psum.tile([128, TC], f32, tag="sum",
                                               name=f"sum{h}_{j}")
                    p, c0 = pt.pop((h, kt))
                    nc.tensor.matmul(acc[h][:, c0:TC], ones_sq[:],
                                     p[:, c0:TC],
                                     start=(kt == 0), stop=(kt == nkt - 1),
                                     skip_group_check=True)
                    nc.tensor.matmul(ps_y[h][:, c0:TC], v_sb[kt][:],
                                     p[:, c0:TC],
                                     start=(kt == 0), stop=(kt == nkt - 1),
                                     skip_group_check=True)
                    if kt != nkt - 1:
                        continue
                    # normalize straight off PSUM: fast approx reciprocal of
                    # the exp-sum, then scale the y accumulator while
                    # converting to bf16 (frees both banks immediately after)
                    rsum = norm_pool.tile([HD, TC], f32, tag="rsum",
                                          name=f"rs{h}_{j}")
                    nc.vector.reciprocal_approx_fast(rsum[:], acc[h][:])
                    yt_t = yt_pool.tile([HD, TC], bf16, tag=f"yt{h}",
                                        name=f"yt{h}_{j}")
                    nc.vector.tensor_mul(yt_t[:], ps_y[h][:], rsum[:])
                    yts[h] = yt_t
            return yts

        # ====== Phase P: partial c_proj over local 512 channels (chunk j) ======
        def phase_p(j, yts, wp):
            # rs_in[j][t, o] = sum_h yt[h]^T @ wp[h]  (contraction over 512 ch)
            seg_starts = []
            s = 0
            for r in rs_rows[j]:
                seg_starts.append(s)
                s += r

            def seg_of(tt):
                for si in reversed(range(len(seg_starts))):
                    if 128 * tt >= seg_starts[si]:
                        return si
                return 0

            for tt in range(NQ):
                if tt > 0 and seg_of(tt) != seg_of(tt - 1):
                    phase_rs(j, seg_of(tt - 1))
                for ob in range(NQ):
                    # later groups borrow the idle attention accumulator
                    # banks so mm_psum drains before the next chunk's
                    # score matmuls need it
                    gi = tt * NQ + ob
                    if gi < 8:
                        ps = mm_psum.tile([128, TC], f32, tag="mm",
                                          name=f"pp{j}_{tt}_{ob}")
                    elif gi < 12:
                        ps = y_psum.tile([HD, TC], f32, tag="y",
                                         name=f"pp{j}_{tt}_{ob}")
                    else:
                        ps = sum_psum.tile([128, TC], f32, tag="sum",
                                           name=f"pp{j}_{tt}_{ob}")
                    for h in range(N_HL):
                        nc.tensor.matmul(
                            ps[:],
                            yts[h][:, 128 * tt:128 * (tt + 1)],
                            wp[h][:, TC * ob:TC * (ob + 1)],
                            start=(h == 0), stop=(h == N_HL - 1))
                    ev = o_sb_pool.tile([128, TC], bf16, tag="ev",
                                        name=f"ev{j}_{tt}_{ob}")
                    # mostly-DVE evictions: ACT eviction work sits ahead of
                    # the next chunk's softmax exps in the ACT queue, so
                    # keep its share small
                    if gi % 4 == 1:
                        nc.scalar.activation(ev[:], ps[:], AF.Copy)
                    else:
                        nc.vector.tensor_copy(ev[:], ps[:])
                    si = seg_of(tt)
                    rr = 128 * tt - seg_starts[si]
                    nc.sync.dma_start(
                        rs_in[j][si][rr:rr + 128, TC * ob:TC * (ob + 1)],
                        ev[:])

        def phase_rs(j, hh):
            # trigger-only: the out-DMA bounce happens in the epilogue so
            # the gpsimd queue never blocks a later collective trigger
            # behind a DMA that waits on an earlier collective
            nc.gpsimd.collective_compute(
                "ReduceScatter",
                mybir.AluOpType.add,
                replica_groups=[[0, 1, 2, 3], [4, 5, 6, 7]],
                ins=[rs_in[j][hh].opt()],
                outs=[rs_out[j][hh].opt()],
            )

        def epilogue():
            # collectives cannot write IO tensors; bounce DRAM->DRAM.
            # Ready pieces (chunks 0..NT-2) first, then the still-in-flight
            # chunk NT-1 pieces, spread over all three queues so each queue
            # ends on at most one waiting DMA.
            qs = [nc.gpsimd, nc.sync, nc.scalar]
            items = []
            for j in range(NT):
                for hh in range(len(rs_rows[j])):
                    seg0 = sum(rs_rows[j][:hh])
                    nrow = rs_rows[j][hh] // G
                    r0 = 128 * j + seg0 // G
                    items.append((j, hh, r0, nrow))
            qi = 0
            for j, hh, r0, nrow in items:
                last = (j == NT - 1) and (hh == len(rs_rows[j]) - 1)
                if not last:
                    qs[qi % 3].dma_start(out[r0:r0 + nrow, :],
                                         rs_out[j][hh][:])
                    qi += 1
                else:
                    nc.sync.dma_start(out[r0:r0 + nrow // 2, :],
                                      rs_out[j][hh][0:nrow // 2, :])
                    nc.scalar.dma_start(out[r0 + nrow // 2:r0 + nrow, :],
                                        rs_out[j][hh][nrow // 2:nrow, :])

        with tc.tile_pool(name="wq_pool", bufs=1) as wq_pool, \
             tc.tile_pool(name="xt_pool", bufs=2) as xt_pool, \
             tc.tile_pool(name="cs_pool", bufs=1) as cs_pool, \
             tc.tile_pool(name="wp_pool", bufs=1) as wp_pool:
            cos_sb = cs_pool.tile([HD, T], f32)
            sin_sb = cs_pool.tile([HD, T], f32)
            wq = [wq_pool.tile([128, NR * HD], bf16, tag=f"wq{ct}",
                               name=f"wq{ct}")
                  for ct in range(NCT)]
            # critical path: first matmuls need wq[ct] + chunk-0 xt[ct] in ct
            # order.  wq rides the two HWDGE queues; chunk-0 xt tiles ride
            # the 4-wide SWDGE path.
            for ct in range(NCT):
                (nc.scalar if ct % 2 == 0 else nc.sync).dma_start(
                    wq[ct][:], wqkvT[128 * ct:128 * (ct + 1), :])
            # RoPE tables: chunk-0 slice right behind wq (first read after
            # the r=0 qkv chain); remaining slices after phase_a(0)
            nc.scalar.dma_start(cos_sb[:, 0:TC], cosT[:, 0:TC])
            nc.sync.dma_start(sin_sb[:, 0:TC], sinT[:, 0:TC])
            wp = [wp_pool.tile([128, C], bf16, tag=f"wp{h}", name=f"wp{h}")
                  for h in range(N_HL)]
            phase_a(0, wq, xt_pool, cos_sb, sin_sb, [nc.gpsimd])
            for j in range(1, NT):
                nc.scalar.dma_start(cos_sb[:, TC * j:TC * (j + 1)],
                                    cosT[:, TC * j:TC * (j + 1)])
                nc.sync.dma_start(sin_sb[:, TC * j:TC * (j + 1)],
                                  sinT[:, TC * j:TC * (j + 1)])
            for h in range(N_HL):
                (nc.scalar if h % 2 == 0 else nc.sync).dma_start(
                    wp[h][:], wpT[128 * h:128 * (h + 1), :])
            nc.gpsimd.dma_start(mask_sb[:], binmask[:])
            if kmask is not None:
                nc.gpsimd.dma_start(kmask_sb[:], kmask[:])
            # Interleave remaining qkv chunks with attention: while B(j)'s
            # matmuls run, A(j+2)'s xt tiles stream in, so qkv is never
            # DMA-throttled (keeps the PE p-state at full clock).
            qs3 = [nc.scalar, nc.sync, nc.gpsimd]

            def do_a(j):
                phase_a(j, wq, xt_pool, cos_sb, sin_sb,
                        [qs3[jj % 3] for jj in range(j, j + 3)])

            def do_bp(j):
                yts = phase_b(j)
                phase_p(j, yts, wp)
                phase_rs(j, len(rs_rows[j]) - 1)

            do_a(1)
            do_bp(0)
            do_a(2)
            do_bp(1)
            do_a(3)
            do_bp(2)
            do_bp(3)
            epilogue()


def _rope_tables():
    inv_freq = 1.0 / (ROPE_THETA ** (np.arange(0, HD, 2, dtype=np.float32) / HD))
    pos = np.arange(T, dtype=np.float32)
    freqs = pos[:, None] * inv_freq[None, :]
    emb = np.concatenate([freqs, freqs], axis=-1)          # [T, HD]
    cos = np.ascontiguousarray(np.cos(emb).astype(np.float32).T)   # [HD, T]
    sin = np.ascontiguousarray(np.sin(emb).astype(np.float32).T)
    sin[:64, :] *= -1.0                                    # sign for rotate_half
    return cos, sin


def _binmask():
    kk = np.arange(128)[:, None]
    qq = np.arange(TC)[None, :]
    blocks = [(kk <= qq - 128 * r).astype(np.float32) for r in range(G)]
    return np.ascontiguousarray(np.concatenate(blocks, axis=1))  # [128, 4*512]


def kernel(x, attention_mask, w_qkv, w_proj):
    from concourse.bass_utils import run_bass_kernel_spmd

    x = np.asarray(x, dtype=np.float32)
    attention_mask = np.asarray(attention_mask, dtype=np.float32)
    w_qkv = np.asarray(w_qkv, dtype=np.float32)
    w_proj = np.asarray(w_proj, dtype=np.float32)

    masked = not bool((attention_mask == 1.0).all())
    if masked:
        attention_mask = (attention_mask != 0.0).astype(np.float32)

    if masked not in _COMPILED:
        _COMPILED[masked] = _build(masked)
    nc = _COMPILED[masked]

    import ml_dtypes
    bf = ml_dtypes.bfloat16
    cos, sin = _rope_tables()
    bm = _binmask().astype(bf)
    wpT_full = np.ascontiguousarray(w_proj.T)              # [c, o]

    in_maps = []
    for c in range(N_CORES):
        b, g = divmod(c, G)
        xT_b = np.ascontiguousarray(x[b].T).astype(bf)
        rows_q = w_qkv[512 * g:512 * (g + 1)]
        rows_k = w_qkv[H * HD + HD * g:H * HD + HD * (g + 1)]
        rows_v = w_qkv[(H + KV) * HD + HD * g:(H + KV) * HD + HD * (g + 1)]
        wqkvT_g = np.ascontiguousarray(
            np.concatenate([rows_q, rows_k, rows_v], axis=0).T).astype(bf)
        m = {"xT": xT_b,
             "wqkvT": wqkvT_g,
             "wpT": np.ascontiguousarray(
                 wpT_full[TC * g:TC * (g + 1), :]).astype(bf),
             "cosT": cos, "sinT": sin, "binmask": bm}
        if masked:
            m["kmask"] = np.ascontiguousarray(
                attention_mask[b].reshape(NKT, 128).T)
        in_maps.append(m)

    trace = bool(globals().get("_TRACE", False))
    res = run_bass_kernel_spmd(nc, in_maps, core_ids=list(range(N_CORES)),
                               trace=trace)
    globals()["_LAST_RESULT"] = res

    y = np.empty((B, T, C), dtype=np.float32)
    for c in range(N_CORES):
        b, g = divmod(c, G)
        o = np.asarray(res.results[c]["out"], dtype=np.float32)  # [512, 2048]
        for j in range(NT):
            segs = RS_SEGS[j]
            s0 = 0
            for seg in segs:
                nrow = seg // G
                r0 = TC * j + s0 + nrow * g
                k0 = 128 * j + s0 // G
                y[b, r0:r0 + nrow, :] = o[k0:k0 + nrow, :]
                s0 += seg
    return y



# revision 43
# speedup vs baseline: 1.1506x; 1.0125x over previous
"""Causal self-attention (GQA + RoPE) on 8 Trainium2 NeuronCores.

Sharding: data-parallel over batch (2) x tensor-parallel over KV-head groups
(4).  Core c handles batch b=c//4 and KV group g=c%4 (Q heads 4g..4g+3).
Each core computes qkv^T in a transposed [d, t] layout, runs attention with
scores in S^T[k, q] orientation (no transposes needed), then computes a
PARTIAL output projection over its local 512 channels for all 2048 output
columns (contraction-sharded c_proj) and ReduceScatters the bf16 partials
per token-segment straight into the output rows it owns.

Schedule/engine notes (each validated against perfetto traces):
- qkv chunks interleave with attention chunks (A0 A1 B0 A2 B1 A3 B2 B3) so
  qkv is never DMA-throttled and the PE p-state stays at full clock.
- softmax normalization runs straight off PSUM: reciprocal_approx_fast on
  the exp-sum accumulator + one tensor_mul, no intermediate copies.
- ReduceScatter is segmented ([256,256] per chunk, [384,128] for the last
  chunk) and triggered as soon as its token rows are evicted; all
  DRAM->DRAM out-bounces are deferred to an epilogue so the gpsimd queue
  never blocks a later collective trigger behind a DMA waiting on an
  earlier collective.
- c_proj PSUM groups borrow the idle attention accumulator banks, and
  evictions go 3:1 DVE:ACT (ACT eviction work would sit ahead of the next
  chunk's softmax exps in the ACT queue).
Matmuls run in fp32r for scores and bf16 elsewhere.
"""

import sys

if "/opt/trn_rl_repo" not in sys.path:
    sys.path.insert(0, "/opt/trn_rl_repo")

import numpy as np

# Problem constants (hardcoded per contract)
B, T, C = 2, 2048, 2048
H, KV, HD = 16, 4, 128
G = 4               # TP groups (KV heads) per batch
N_CORES = 8
N_HL = H // KV      # local Q heads per core = 4
ROPE_THETA = 10000.0
TC = 512            # free-dim chunk for moving operands
NT = T // TC        # 4 t-chunks
# ReduceScatter row-segments per chunk (last chunk ends with a small piece
# so the final serialized collective+DMA tail is short)
RS_SEGS = [[256, 256], [256, 256], [256, 256], [384, 128]]
NCT = C // 128      # 16 contraction tiles
NKT = T // 128      # 16 key tiles
SCALE = float(1.0 / np.sqrt(HD))

_COMPILED = {}


def _build(masked: bool):
    import concourse.bacc as bacc
    import concourse.tile as tile
    import concourse.mybir as mybir

    f32 = mybir.dt.float32

    nc = bacc.Bacc("TRN2", target_bir_lowering=False, debug=False,
                   num_devices=N_CORES, num_swdge_queues=4)

    bf16 = mybir.dt.bfloat16
    xT = nc.dram_tensor("xT", [C, T], bf16, kind="ExternalInput").ap()
    wqkvT = nc.dram_tensor("wqkvT", [C, (N_HL + 2) * HD], bf16,
                           kind="ExternalInput").ap()
    # local channel rows of w_proj^T: [512 ch, 2048 out]
    wpT = nc.dram_tensor("wpT", [TC, C], bf16, kind="ExternalInput").ap()
    cosT = nc.dram_tensor("cosT", [HD, T], f32, kind="ExternalInput").ap()
    sinT = nc.dram_tensor("sinT", [HD, T], f32, kind="ExternalInput").ap()
    binmask = nc.dram_tensor("binmask", [128, G * TC], bf16,
                             kind="ExternalInput").ap()
    kmask = None
    if masked:
        kmask = nc.dram_tensor("kmask", [128, NKT], f32,
                               kind="ExternalInput").ap()
    # 4 chunks x 128 token rows owned by this core, all 2048 columns
    out = nc.dram_tensor("out", [NT * 128, C], bf16, kind="ExternalOutput").ap()

    with tile.TileContext(nc, pool_alloc_mode="queue") as tc, \
         nc.allow_low_precision(reason="fp32r intermediates are intentional"):
        _build_body(nc, tc, mybir, f32,
                    xT, wqkvT, wpT, cosT, sinT, binmask, kmask, out)
    nc.compile()
    return nc


def _build_body(nc, tc, mybir, f32,
                xT, wqkvT, wpT, cosT, sinT, binmask, kmask, out):
    bf16 = mybir.dt.bfloat16
    f32r = mybir.dt.float32r
    from contextlib import ExitStack
    from concourse.masks import make_identity

    AF = mybir.ActivationFunctionType
    NR = N_HL + 2  # 6 row-tiles of qkv^T (4 q heads, k, v)
    NQ = TC // 128  # 4 128-subtiles per chunk

    with ExitStack() as ctx:
        # ---- pools (all share the kernel lifetime; queue allocator) ----
        const = ctx.enter_context(tc.tile_pool(name="const", bufs=1))
        rqkv = ctx.enter_context(tc.tile_pool(name="rqkv", bufs=1))
        dram = ctx.enter_context(tc.tile_pool(name="dram", bufs=1, space="DRAM"))
        raw_pool = ctx.enter_context(tc.tile_pool(name="raw_pool", bufs=3))
        rope_tmp = ctx.enter_context(tc.tile_pool(name="rope_tmp", bufs=3))
        pt_pool = ctx.enter_context(tc.tile_pool(name="pt_pool", bufs=8))
        norm_pool = ctx.enter_context(tc.tile_pool(name="norm_pool", bufs=3))
        yt_pool = ctx.enter_context(tc.tile_pool(name="yt_pool", bufs=2))
        o_sb_pool = ctx.enter_context(tc.tile_pool(name="o_sb", bufs=6))
        # PSUM: 4 + 2 + 2 = 8 banks
        mm_psum = ctx.enter_context(
            tc.tile_pool(name="mm_psum", bufs=4, space="PSUM"))
        y_psum = ctx.enter_context(
            tc.tile_pool(name="y_psum", bufs=2, space="PSUM"))
        sum_psum = ctx.enter_context(
            tc.tile_pool(name="sum_psum", bufs=2, space="PSUM"))

        ident = const.tile([128, 128], f32)
        make_identity(nc, ident[:])
        ones_sq_f = const.tile([128, 128], f32)
        nc.vector.memset(ones_sq_f[:], 1.0)
        ones_sq = const.tile([128, 128], bf16)
        nc.vector.tensor_copy(ones_sq[:], ones_sq_f[:])
        # mask tiles allocated here; DMAs issued after the critical-path
        # loads (first needed at phase_b(0), ~100us in)
        mask_sb = const.tile([128, G * TC], bf16)
        kmask_sb = None
        if kmask is not None:
            kmask_sb = const.tile([128, NKT], f32)

        # resident activations (rotated q/k in [d, t] layout, v in [t, d])
        rq = [rqkv.tile([HD, T], f32r, tag=f"rq{h}", name=f"rq{h}")
              for h in range(N_HL)]
        rk = rqkv.tile([HD, T], f32r, tag="rk")
        v_sb = [rqkv.tile([128, HD], bf16, tag=f"v{i}", name=f"v{i}")
                for i in range(NKT)]

        # ReduceScatter staging, one pair per c_proj token slice.  Chunks
        # 0-2 use two halves [256, C]; the last chunk splits the second half
        # into two quarters [128, C] so the final (fully serialized) RS and
        # out-DMA are as small as possible.
        def rs_tiles(j):
            rows = RS_SEGS[j]
            ins, outs = [], []
            for hh, r in enumerate(rows):
                ins.append(dram.tile([r, C], bf16, tag=f"rsi{j}_{hh}",
                                     name=f"rsi{j}_{hh}"))
                outs.append(dram.tile([r // G, C], bf16, tag=f"rso{j}_{hh}",
                                      name=f"rso{j}_{hh}"))
            return ins, outs, rows
        _rs = [rs_tiles(j) for j in range(NT)]
        rs_in = [t[0] for t in _rs]
        rs_out = [t[1] for t in _rs]
        rs_rows = [t[2] for t in _rs]

        # ============ Phase A: qkv^T projection + RoPE + V transpose ============
        def phase_a(j, wq, xt_pool, cos_sb, sin_sb, xt_qs, pre=None):
            if pre is not None:
                xt_tiles = pre
            else:
                xt_tiles = []
                for ct in range(NCT):
                    xt_t = xt_pool.tile([128, TC], bf16, tag=f"xt{ct}",
                                        name=f"xt{ct}_{j}")
                    xt_qs[ct % len(xt_qs)].dma_start(
                        xt_t[:], xT[128 * ct:128 * (ct + 1),
                                    TC * j:TC * (j + 1)])
                    xt_tiles.append(xt_t)
            for r in range(NR):
                ps = mm_psum.tile([128, TC], f32, tag="mm", name=f"qkv{r}_{j}")
                for ct in range(NCT):
                    nc.tensor.matmul(ps[:],
                                     wq[ct][:, 128 * r:128 * (r + 1)],
                                     xt_tiles[ct][:],
                                     start=(ct == 0), stop=(ct == NCT - 1))
                if r < N_HL + 1:
                    # RoPE straight off PSUM:
                    #   dst = ps*cos + rot_half(ps)*sin_signed
                    dst = (rq[r] if r < N_HL else rk)[:, TC * j:TC * (j + 1)]
                    cs = cos_sb[:, TC * j:TC * (j + 1)]
                    sn = sin_sb[:, TC * j:TC * (j + 1)]
                    t1 = rope_tmp.tile([128, TC], f32, tag="t1",
                                       name=f"t1_{r}_{j}")
                    nc.vector.tensor_mul(t1[:], ps[:], cs[:])
                    t2 = rope_tmp.tile([128, TC], f32, tag="t2",
                                       name=f"t2_{r}_{j}")
                    nc.vector.tensor_mul(t2[0:64, :], ps[64:128, :], sn[0:64, :])
                    nc.vector.tensor_mul(t2[64:128, :], ps[0:64, :], sn[64:128, :])
                    nc.vector.tensor_add(dst, t1[:], t2[:])
                else:
                    # V: evict then transpose v^T [d, t] -> v [t, d]
                    rt = raw_pool.tile([128, TC], f32, tag="raw",
                                       name=f"vraw{j}")
                    nc.vector.tensor_copy(rt[:], ps[:])
                    for q in range(NQ):
                        pt = mm_psum.tile([128, 128], f32, tag="mm",
                                          name=f"vt{j}_{q}")
                        nc.tensor.transpose(
                            pt[:], rt[:, 128 * q:128 * (q + 1)], ident[:])
                        nc.vector.tensor_copy(v_sb[j * NQ + q][:], pt[:])

        # ============ Phase B: attention for q-chunk j ============
        def phase_b(j):
            nkt = (j + 1) * NQ  # causal limit in 128-k tiles
            ps_y = {}
            acc = {}
            pt = {}
            yts = {}

            def emit_scores(h, kt):
                # diagonal k-tile r>0: queries q < 128r in this chunk lie
                # strictly above the causal boundary -> p==0 there; skip
                # computing those columns entirely.
                r = kt - NQ * j
                c0 = 128 * r if r > 0 else 0
                ps_s = mm_psum.tile([128, TC], f32, tag="mm",
                                    name=f"s{h}_{j}_{kt}")
                nc.tensor.matmul(ps_s[:, c0:TC],
                                 rk[:, 128 * kt:128 * (kt + 1)],
                                 rq[h][:, TC * j + c0:TC * (j + 1)],
                                 start=True, stop=True)
                p = pt_pool.tile([128, TC], bf16, tag="pt",
                                 name=f"pt{h}_{j}_{kt}")
                nc.scalar.activation(p[:, c0:TC], ps_s[:, c0:TC],
                                     AF.Exp, scale=SCALE)
                if r >= 0:
                    nc.vector.tensor_mul(
                        p[:, c0:TC], p[:, c0:TC],
                        mask_sb[:, TC * r + c0:TC * (r + 1)])
                if kmask_sb is not None:
                    nc.vector.tensor_scalar_mul(
                        p[:, c0:TC], p[:, c0:TC], kmask_sb[:, kt:kt + 1])
                pt[(h, kt)] = (p, c0)

            for hp in range(N_HL // 2):
                pair = (2 * hp, 2 * hp + 1)
                units = [(h, kt) for kt in range(nkt) for h in pair]
                emit_scores(*units[0])
                if len(units) > 1:
                    emit_scores(*units[1])
                for idx, (h, kt) in enumerate(units):
                    if idx + 2 < len(units):
                        emit_scores(*units[idx + 2])
                    if kt == 0:
                        ps_y[h] = y_psum.tile([HD, TC], f32, tag="y",
                                              name=f"y{h}_{j}")
                        acc[h] = sum_psum.tile([128, TC], f32, tag="sum",
                                               name=f"sum{h}_{j}")
                    p, c0 = pt.pop((h, kt))
                    nc.tensor.matmul(acc[h][:, c0:TC], ones_sq[:],
                                     p[:, c0:TC],
                                     start=(kt == 0), stop=(kt == nkt - 1),
                                     skip_group_check=True)
                    nc.tensor.matmul(ps_y[h][:, c0:TC], v_sb[kt][:],
                                     p[:, c0:TC],
                                     start=(kt == 0), stop=(kt == nkt - 1),
                                     skip_group_check=True)
                    if kt != nkt - 1:
                        continue
                    # normalize straight off PSUM: fast approx reciprocal of
                    # the exp-sum, then scale the y accumulator while
                    # converting to bf16 (frees both banks immediately after)
                    rsum = norm_pool.tile([HD, TC], f32, tag="rsum",
                                          name=f"rs{h}_{j}")
                    nc.vector.reciprocal_approx_fast(rsum[:], acc[h][:])
                    yt_t = yt_pool.tile([HD, TC], bf16, tag=f"yt{h}",
                                        name=f"yt{h}_{j}")
                    nc.vector.tensor_mul(yt_t[:], ps_y[h][:], rsum[:])
                    yts[h] = yt_t
            return yts

        # ====== Phase P: partial c_proj over local 512 channels (chunk j) ======
        def phase_p(j, yts, wp):
            # rs_in[j][t, o] = sum_h yt[h]^T @ wp[h]  (contraction over 512 ch)
            seg_starts = []
            s = 0
            for r in rs_rows[j]:
                seg_starts.append(s)
                s += r

            def seg_of(tt):
                for si in reversed(range(len(seg_starts))):
                    if 128 * tt >= seg_starts[si]:
                        return si
                return 0

            for tt in range(NQ):
                if tt > 0 and seg_of(tt) != seg_of(tt - 1):
                    phase_rs(j, seg_of(tt - 1))
                for ob in range(NQ):
                    # later groups borrow the idle attention accumulator
                    # banks so mm_psum drains before the next chunk's
                    # score matmuls need it
                    gi = tt * NQ + ob
                    if gi < 8:
                        ps = mm_psum.tile([128, TC], f32, tag="mm",
                                          name=f"pp{j}_{tt}_{ob}")
                    elif gi < 12:
                        ps = y_psum.tile([HD, TC], f32, tag="y",
                                         name=f"pp{j}_{tt}_{ob}")
                    else:
                        ps = sum_psum.tile([128, TC], f32, tag="sum",
                                           name=f"pp{j}_{tt}_{ob}")
                    for h in range(N_HL):
                        nc.tensor.matmul(
                            ps[:],
                            yts[h][:, 128 * tt:128 * (tt + 1)],
                            wp[h][:, TC * ob:TC * (ob + 1)],
                            start=(h == 0), stop=(h == N_HL - 1))
                    ev = o_sb_pool.tile([128, TC], bf16, tag="ev",
                                        name=f"ev{j}_{tt}_{ob}")
                    # mostly-DVE evictions: ACT eviction work sits ahead of
                    # the next chunk's softmax exps in the ACT queue, so
                    # keep its share small
                    if gi % 4 == 1:
                        nc.scalar.activation(ev[:], ps[:], AF.Copy)
                    else:
                        nc.vector.tensor_copy(ev[:], ps[:])
                    si = seg_of(tt)
                    rr = 128 * tt - seg_starts[si]
                    nc.sync.dma_start(
                        rs_in[j][si][rr:rr + 128, TC * ob:TC * (ob + 1)],
                        ev[:])

        def phase_rs(j, hh):
            # trigger-only: the out-DMA bounce happens in the epilogue so
            # the gpsimd queue never blocks a later collective trigger
            # behind a DMA that waits on an earlier collective
            nc.gpsimd.collective_compute(
                "ReduceScatter",
                mybir.AluOpType.add,
                replica_groups=[[0, 1, 2, 3], [4, 5, 6, 7]],
                ins=[rs_in[j][hh].opt()],
                outs=[rs_out[j][hh].opt()],
            )

        def epilogue():
            # collectives cannot write IO tensors; bounce DRAM->DRAM.
            # Ready pieces (chunks 0..NT-2) first, then the still-in-flight
            # chunk NT-1 pieces, spread over all three queues so each queue
            # ends on at most one waiting DMA.
            qs = [nc.gpsimd, nc.sync, nc.scalar]
            items = []
            for j in range(NT):
                for hh in range(len(rs_rows[j])):
                    seg0 = sum(rs_rows[j][:hh])
                    nrow = rs_rows[j][hh] // G
                    r0 = 128 * j + seg0 // G
                    items.append((j, hh, r0, nrow))
            qi = 0
            for j, hh, r0, nrow in items:
                last = (j == NT - 1) and (hh == len(rs_rows[j]) - 1)
                if not last:
                    qs[qi % 3].dma_start(out[r0:r0 + nrow, :],
                                         rs_out[j][hh][:])
                    qi += 1
                else:
                    nc.sync.dma_start(out[r0:r0 + nrow // 2, :],
                                      rs_out[j][hh][0:nrow // 2, :])
                    nc.scalar.dma_start(out[r0 + nrow // 2:r0 + nrow, :],
                                        rs_out[j][hh][nrow // 2:nrow, :])

        with tc.tile_pool(name="wq_pool", bufs=1) as wq_pool, \
             tc.tile_pool(name="xt_pool", bufs=2) as xt_pool, \
             tc.tile_pool(name="cs_pool", bufs=1) as cs_pool, \
             tc.tile_pool(name="wp_pool", bufs=1) as wp_pool:
            cos_sb = cs_pool.tile([HD, T], f32)
            sin_sb = cs_pool.tile([HD, T], f32)
            wq = [wq_pool.tile([128, NR * HD], bf16, tag=f"wq{ct}",
                               name=f"wq{ct}")
                  for ct in range(NCT)]
            # critical path: first matmuls need wq[ct] + chunk-0 xt[ct] in ct
            # order.  wq rides the two HWDGE queues; chunk-0 xt tiles ride
            # the 4-wide SWDGE path.
            for ct in range(NCT):
                (nc.scalar if ct % 2 == 0 else nc.sync).dma_start(
                    wq[ct][:], wqkvT[128 * ct:128 * (ct + 1), :])
            # RoPE tables: chunk-0 slice right behind wq (first read after
            # the r=0 qkv chain); remaining slices after phase_a(0)
            nc.scalar.dma_start(cos_sb[:, 0:TC], cosT[:, 0:TC])
            nc.sync.dma_start(sin_sb[:, 0:TC], sinT[:, 0:TC])
            wp = [wp_pool.tile([128, C], bf16, tag=f"wp{h}", name=f"wp{h}")
                  for h in range(N_HL)]
            phase_a(0, wq, xt_pool, cos_sb, sin_sb, [nc.gpsimd])
            for j in range(1, NT):
                nc.scalar.dma_start(cos_sb[:, TC * j:TC * (j + 1)],
                                    cosT[:, TC * j:TC * (j + 1)])
                nc.sync.dma_start(sin_sb[:, TC * j:TC * (j + 1)],
                                  sinT[:, TC * j:TC * (j + 1)])
            for h in range(N_HL):
                (nc.scalar if h % 2 == 0 else nc.sync).dma_start(
                    wp[h][:], wpT[128 * h:128 * (h + 1), :])
            nc.gpsimd.dma_start(mask_sb[:], binmask[:])
            if kmask is not None:
                nc.gpsimd.dma_start(kmask_sb[:], kmask[:])
            # Interleave remaining qkv chunks with attention: while B(j)'s
            # matmuls run, A(j+2)'s xt tiles stream in, so qkv is never
            # DMA-throttled (keeps the PE p-state at full clock).
            qs3 = [nc.scalar, nc.sync, nc.gpsimd]

            def do_a(j):
                phase_a(j, wq, xt_pool, cos_sb, sin_sb,
                        [qs3[jj % 3] for jj in range(j, j + 3)])

            def do_bp(j):
                yts = phase_b(j)
                phase_p(j, yts, wp)
                phase_rs(j, len(rs_rows[j]) - 1)

            do_a(1)
            do_bp(0)
            do_a(2)
            do_bp(1)
            do_a(3)
            do_bp(2)
            do_bp(3)
            epilogue()


def _rope_tables():
    inv_freq = 1.0 / (ROPE_THETA ** (np.arange(0, HD, 2, dtype=np.float32) / HD))
    pos = np.arange(T, dtype=np.float32)
    freqs = pos[:, None] * inv_freq[None, :]
    emb = np.concatenate([freqs, freqs], axis=-1)          # [T, HD]
    cos = np.ascontiguousarray(np.cos(emb).astype(np.float32).T)   # [HD, T]
    sin = np.ascontiguousarray(np.sin(emb).astype(np.float32).T)
    sin[:64, :] *= -1.0                                    # sign for rotate_half
    return cos, sin


def _binmask():
    kk = np.arange(128)[:, None]
    qq = np.arange(TC)[None, :]
    blocks = [(kk <= qq - 128 * r).astype(np.float32) for r in range(G)]
    return np.ascontiguousarray(np.concatenate(blocks, axis=1))  # [128, 4*512]


def kernel(x, attention_mask, w_qkv, w_proj):
    from concourse.bass_utils import run_bass_kernel_spmd

    x = np.asarray(x, dtype=np.float32)
    attention_mask = np.asarray(attention_mask, dtype=np.float32)
    w_qkv = np.asarray(w_qkv, dtype=np.float32)
    w_proj = np.asarray(w_proj, dtype=np.float32)

    masked = not bool((attention_mask == 1.0).all())
    if masked:
        attention_mask = (attention_mask != 0.0).astype(np.float32)

    if masked not in _COMPILED:
        _COMPILED[masked] = _build(masked)
    nc = _COMPILED[masked]

    import ml_dtypes
    bf = ml_dtypes.bfloat16
    cos, sin = _rope_tables()
    bm = _binmask().astype(bf)
    wpT_full = np.ascontiguousarray(w_proj.T)              # [c, o]

    in_maps = []
    for c in range(N_CORES):
        b, g = divmod(c, G)
        xT_b = np.ascontiguousarray(x[b].T).astype(bf)
        rows_q = w_qkv[512 * g:512 * (g + 1)]
        rows_k = w_qkv[H * HD + HD * g:H * HD + HD * (g + 1)]
        rows_v = w_qkv[(H + KV) * HD + HD * g:(H + KV) * HD + HD * (g + 1)]
        wqkvT_g = np.ascontiguousarray(
            np.concatenate([rows_q, rows_k, rows_v], axis=0).T).astype(bf)
        m = {"xT": xT_b,
             "wqkvT": wqkvT_g,
             "wpT": np.ascontiguousarray(
                 wpT_full[TC * g:TC * (g + 1), :]).astype(bf),
             "cosT": cos, "sinT": sin, "binmask": bm}
        if masked:
            m["kmask"] = np.ascontiguousarray(
                attention_mask[b].reshape(NKT, 128).T)
        in_maps.append(m)

    trace = bool(globals().get("_TRACE", False))
    res = run_bass_kernel_spmd(nc, in_maps, core_ids=list(range(N_CORES)),
                               trace=trace)
    globals()["_LAST_RESULT"] = res

    y = np.empty((B, T, C), dtype=np.float32)
    for c in range(N_CORES):
        b, g = divmod(c, G)
        o = np.asarray(res.results[c]["out"], dtype=np.float32)  # [512, 2048]
        for j in range(NT):
            segs = RS_SEGS[j]
            s0 = 0
            for seg in segs:
                nrow = seg // G
                r0 = TC * j + s0 + nrow * g
                k0 = 128 * j + s0 // G
                y[b, r0:r0 + nrow, :] = o[k0:k0 + nrow, :]
                s0 += seg
    return y

